# revision 21
# baseline (speedup 1.0000x reference)
# Tensor-parallel Trainium2 kernel for AttnDecoderRNN single step.
#
# 8-way tensor parallel: every weight matrix is sharded over its output dim
# (H slices of 128 for the small chain, vocab slices of 6250 for out_w/out_b).
# Cross-core glue: AllReduce(scores[7]) -> AllGather(x[128]) ->
# AllGather(h1[128]) -> AllReduce(sum_exp[1]) for the log_softmax.
#
# Host side only slices/transposes inputs into DMA-friendly per-core images
# (plus the emb row gather, which is pure input sharding) and concatenates the
# per-core outputs.

import numpy as np
from contextlib import ExitStack

NCORES = 8
H = 1024
V = 50000
T = 7
S = 128              # H slice per core
VS = V // NCORES     # 6250 vocab rows per core
VTILES = 49          # ceil(VS / 128)
VPAD = VTILES * 128  # 6272
KH = H // 128        # 8 k-tiles over H
NEG_BIG = -1.0e30

# stage D chunking: 7 chunks x 7 vocab tiles
D_CHUNK_VT = 7
D_NCHUNKS = VTILES // D_CHUNK_VT  # 7
D_CHUNK_COLS = D_CHUNK_VT * KH * 128  # 7168


def _img_lhsT(w_mk: np.ndarray) -> np.ndarray:
    """[M, K] weight (out = W @ v) -> SBUF image [128, KT*M] of lhsT tiles.

    img[p, k*M + m] = w_mk[m, k*128 + p]; lhsT tile k = img[:, k*M:(k+1)*M].
    """
    m, k = w_mk.shape
    assert k % 128 == 0
    w3 = w_mk.reshape(m, k // 128, 128)      # [m, k, p]
    return np.ascontiguousarray(w3.transpose(2, 1, 0).reshape(128, (k // 128) * m),
                                dtype=np.float32)


def _img_vec(v: np.ndarray) -> np.ndarray:
    """[n*128] vector -> image [128, n] with img[p, k] = v[k*128 + p]."""
    n = v.shape[0] // 128
    return np.ascontiguousarray(v.reshape(n, 128).T, dtype=np.float32)


def prep_in_maps(inputs: dict) -> list[dict]:
    """Full (unsharded) numpy inputs -> per-core input dicts."""
    f = lambda x: np.asarray(x, dtype=np.float32)
    i32 = lambda x: np.asarray(x, dtype=np.int32)

    input_id = i32(inputs["input_id"])
    attr = i32(inputs["attr"])
    hidden = f(inputs["hidden"])
    cell = f(inputs["cell"])
    enc = f(inputs["encoder_outputs"])
    emb = f(inputs["emb"])
    wk_emb = f(inputs["wk_emb"])
    dd_emb = f(inputs["dd_emb"])
    WD_w = f(inputs["WD_w"])
    WD_b = f(inputs["WD_b"])
    UD_w = f(inputs["UD_w"])
    UD_b = f(inputs["UD_b"])
    VD_w = f(inputs["VD_w"])
    VD_b = f(inputs["VD_b"])
    AC_w = f(inputs["AC_w"])
    AC_b = f(inputs["AC_b"])
    Wih = f(inputs["Wih"])
    Whh = f(inputs["Whh"])
    bih = f(inputs["bih"])
    bhh = f(inputs["bhh"])
    out_w = f(inputs["out_w"])
    out_b = f(inputs["out_b"])

    h0 = hidden.reshape(H)
    c0 = cell.reshape(H)

    # embedding gathers (input sharding: ship only the rows we need)
    embedded = emb[int(input_id[0])]           # [H]
    weekday = wk_emb[int(attr[0])]             # [10]
    deltaday = dd_emb[int(attr[1])]            # [10]

    # emb_comb (1044) padded to 1152 = 9*128
    emb_comb = np.zeros(9 * 128, dtype=np.float32)
    emb_comb[:H] = embedded
    emb_comb[H:H + 10] = weekday
    emb_comb[H + 10:H + 20] = deltaday
    emb_img = _img_vec(emb_comb)               # [128, 9]

    # AC_w permuted so device-side e vector = [ctx(1024); emb_comb_pad(1152)]
    AC_perm = np.zeros((H, 2176), dtype=np.float32)
    AC_perm[:, 0:1024] = AC_w[:, 1044:2068]    # ctx part
    AC_perm[:, 1024:1024 + 1044] = AC_w[:, 0:1044]  # embedded/wk/dd part

    ht = np.concatenate([h0, c0])              # [2048]
    ht_img = _img_vec(ht)                      # [128, 16]
    h0_img = _img_vec(h0)                      # [128, 8]

    encT_img = np.ascontiguousarray(
        enc.T.reshape(KH, 128, T).transpose(1, 0, 2).reshape(128, KH * T),
        dtype=np.float32)                      # [128, 56]

    b_gates_full = bih + bhh                   # [4096]

    in_maps = []
    for j in range(NCORES):
        sl = slice(j * S, (j + 1) * S)

        # LSTM shard: rows for gate slices i,f,g,o of this core's H slice
        rows = np.concatenate([np.arange(g * H + j * S, g * H + (j + 1) * S)
                               for g in range(4)])
        M_j = np.concatenate([Wih[rows], Whh[rows]], axis=1)  # [512, 2048]

        # out_w shard, padded to 6272 rows, stationary-tile image
        Wsh = np.zeros((VPAD, H), dtype=np.float32)
        Wsh[:VS] = out_w[j * VS:(j + 1) * VS]
        w4 = Wsh.reshape(VTILES, 128, KH, 128)                # [vt, m, k, p]
        w_img = np.ascontiguousarray(
            w4.transpose(3, 0, 2, 1).reshape(128, VTILES * KH * 128),
            dtype=np.float32)                                 # [128, 50176]

        outb = np.full(VPAD, NEG_BIG, dtype=np.float32)
        outb[:VS] = out_b[j * VS:(j + 1) * VS]
        outb_img = _img_vec(outb)                             # [128, 49]

        in_maps.append({
            "wd_lhsT": _img_lhsT(WD_w[sl]),                   # [128, 2048]
            "ud_lhsT": _img_lhsT(UD_w[sl]),                   # [128, 1024]
            "encT_img": encT_img,
            "ht_img": ht_img,
            "ba_col": np.ascontiguousarray((WD_b[sl] + UD_b[sl]).reshape(S, 1)),
            "vd_col": np.ascontiguousarray(VD_w[0, sl].reshape(S, 1)),
            "vdb": np.ascontiguousarray(VD_b.reshape(1, 1)),
            "ac_lhsT": _img_lhsT(AC_perm[sl]),                # [128, 2176]
            "acb_col": np.ascontiguousarray(AC_b[sl].reshape(S, 1)),
            "emb_img": emb_img,
            "h0_img": h0_img,
            "m_lhsT": _img_lhsT(M_j),                         # [128, 8192]
            "bg_row": np.ascontiguousarray(b_gates_full[rows].reshape(1, 512)),
            "c0_row": np.ascontiguousarray(c0[sl].reshape(1, S)),
            "w_img": w_img,
            "outb_img": outb_img,
        })
    return in_maps


_INPUT_SPECS = [
    ("wd_lhsT", [128, 2048]), ("ud_lhsT", [128, 1024]), ("encT_img", [128, 56]),
    ("ht_img", [128, 16]), ("ba_col", [128, 1]), ("vd_col", [128, 1]),
    ("vdb", [1, 1]), ("ac_lhsT", [128, 2176]), ("acb_col", [128, 1]),
    ("emb_img", [128, 9]), ("h0_img", [128, 8]), ("m_lhsT", [128, 8192]),
    ("bg_row", [1, 512]), ("c0_row", [1, 128]),
    ("w_img", [128, VTILES * KH * 128]), ("outb_img", [128, VTILES]),
]

_module_cache = None


def build_module(debug: bool = False, stage: str = "full"):
    import concourse.bass as bass
    import concourse.mybir as mybir
    import concourse.tile as tile
    from concourse import bacc
    from concourse.tile_rust import add_dep_helper

    f32 = mybir.dt.float32
    AF = mybir.ActivationFunctionType
    ALU = mybir.AluOpType
    AX = mybir.AxisListType

    nc = bacc.Bacc("TRN2", target_bir_lowering=False, debug=debug,
                   num_devices=NCORES)

    ins = {name: nc.dram_tensor(name, shape, f32, kind="ExternalInput")
           for name, shape in _INPUT_SPECS}

    z_out = nc.dram_tensor("z_out", [128, VTILES], f32, kind="ExternalOutput")
    h1_out = nc.dram_tensor("h1_out", [1, S], f32, kind="ExternalOutput")
    c1_out = nc.dram_tensor("c1_out", [1, S], f32, kind="ExternalOutput")
    attn_out = nc.dram_tensor("attn_out", [1, T], f32, kind="ExternalOutput")

    # collective bounce buffers (internal DRAM; outputs Shared)
    cc_s_in = nc.dram_tensor("cc_s_in", [1, 8], f32)
    cc_s_out = nc.dram_tensor("cc_s_out", [1, 8], f32, addr_space="Shared")
    cc_x_in = nc.dram_tensor("cc_x_in", [1, S], f32)
    cc_x_out = nc.dram_tensor("cc_x_out", [NCORES, S], f32, addr_space="Shared")
    cc_h_in = nc.dram_tensor("cc_h_in", [1, S], f32)
    cc_h_out = nc.dram_tensor("cc_h_out", [NCORES, S], f32, addr_space="Shared")
    cc_z_in = nc.dram_tensor("cc_z_in", [1, 8], f32)
    cc_z_out = nc.dram_tensor("cc_z_out", [1, 8], f32, addr_space="Shared")

    groups = [list(range(NCORES))]

    with tile.TileContext(nc) as tc, ExitStack() as ctx:
        _emit_body(nc, tc, ctx, ins, z_out, h1_out, c1_out, attn_out,
                   cc_s_in, cc_s_out, cc_x_in, cc_x_out, cc_h_in, cc_h_out,
                   cc_z_in, cc_z_out, groups, stage)

    nc.compile()
    return nc


def _emit_body(nc, tc, ctx, ins, z_out, h1_out, c1_out, attn_out,
               cc_s_in, cc_s_out, cc_x_in, cc_x_out, cc_h_in, cc_h_out,
               cc_z_in, cc_z_out, groups, stage):
    import concourse.mybir as mybir
    from concourse.tile_rust import add_dep_helper

    f32 = mybir.dt.float32
    AF = mybir.ActivationFunctionType
    ALU = mybir.AluOpType
    AX = mybir.AxisListType

    if True:
        small = ctx.enter_context(tc.tile_pool(name="small", bufs=1))
        work = ctx.enter_context(tc.tile_pool(name="work", bufs=2))
        wpool = ctx.enter_context(tc.tile_pool(name="wpool", bufs=4))
        psum = ctx.enter_context(tc.tile_pool(name="psum", bufs=4, space="PSUM"))
        psz_pool = ctx.enter_context(tc.tile_pool(name="psz", bufs=2, space="PSUM"))

        dma = nc.sync.dma_start

        # ---- phase A: attention ----
        ht_sb = small.tile([128, 16], f32)
        dma(out=ht_sb, in_=ins["ht_img"][:, :])
        wdw_sb = small.tile([128, 2048], f32)
        dma(out=wdw_sb, in_=ins["wd_lhsT"][:, :])
        udw_sb = small.tile([128, 1024], f32)
        dma(out=udw_sb, in_=ins["ud_lhsT"][:, :])
        encT_sb = small.tile([128, KH * T], f32)
        dma(out=encT_sb, in_=ins["encT_img"][:, :])
        ba_sb = small.tile([128, 1], f32)
        dma(out=ba_sb, in_=ins["ba_col"][:, :])
        vd_sb = small.tile([128, 1], f32)
        dma(out=vd_sb, in_=ins["vd_col"][:, :])
        vdb_sb = small.tile([1, 1], f32)
        dma(out=vdb_sb, in_=ins["vdb"][:, :])

        ps_wd = psum.tile([128, 1], f32, tag="ps_small")
        for k in range(16):
            nc.tensor.matmul(ps_wd, wdw_sb[:, k * 128:(k + 1) * 128],
                             ht_sb[:, k:k + 1], start=(k == 0), stop=(k == 15))
        wdb_sb = work.tile([128, 1], f32)
        nc.vector.tensor_add(wdb_sb, ps_wd, ba_sb)

        ps_eh = psum.tile([128, T], f32, tag="ps_small")
        for k in range(KH):
            nc.tensor.matmul(ps_eh, udw_sb[:, k * 128:(k + 1) * 128],
                             encT_sb[:, k * T:(k + 1) * T],
                             start=(k == 0), stop=(k == KH - 1))
        s_sb = work.tile([128, T], f32)
        nc.scalar.activation(out=s_sb, in_=ps_eh, func=AF.Tanh, bias=wdb_sb)

        ps_sc = psum.tile([1, T], f32, tag="ps_small")
        nc.tensor.matmul(ps_sc, vd_sb, s_sb, start=True, stop=True)
        sc_sb = work.tile([1, 8], f32)
        nc.vector.memset(sc_sb, 0.0)
        nc.vector.tensor_scalar_add(sc_sb[:, 0:T], ps_sc, vdb_sb)

        d_in = dma(out=cc_s_in[:, :], in_=sc_sb)
        cc1 = nc.gpsimd.collective_compute(
            "AllReduce", ALU.add, replica_groups=groups,
            ins=[cc_s_in.ap()], outs=[cc_s_out.ap()])
        add_dep_helper(cc1.ins, d_in.ins, reason="cc1 wait scores dma")
        scg_sb = work.tile([1, 8], f32)
        d_out = dma(out=scg_sb, in_=cc_s_out[:, :])
        add_dep_helper(d_out.ins, cc1.ins, reason="scores gather after cc1")

        # softmax over 7 scores (row layout)
        mx = work.tile([1, 1], f32)
        nc.vector.reduce_max(out=mx, in_=scg_sb[:, 0:T], axis=AX.X)
        negmx = work.tile([1, 1], f32)
        nc.vector.tensor_scalar_mul(negmx, mx, -1.0)
        esum = work.tile([1, 1], f32)
        e_sb = work.tile([1, T], f32)
        nc.scalar.activation(out=e_sb, in_=scg_sb[:, 0:T], func=AF.Exp,
                             bias=negmx, accum_out=esum)
        rs = work.tile([1, 1], f32)
        nc.vector.reciprocal(out=rs, in_=esum)
        attn_sb = work.tile([1, T], f32)
        nc.vector.tensor_scalar_mul(attn_sb, e_sb, rs)
        dma(out=attn_out[:, :], in_=attn_sb)
        if stage == "A":
            return

        # broadcast attn over partitions: ones[1,128].T @ attn[1,7]
        ones_row = small.tile([1, 128], f32)
        nc.vector.memset(ones_row, 1.0)
        ps_ab = psum.tile([128, T], f32, tag="ps_small")
        nc.tensor.matmul(ps_ab, ones_row, attn_sb, start=True, stop=True)
        attn_b = work.tile([128, T], f32)
        nc.vector.tensor_copy(attn_b, ps_ab)
        if stage == "A2":
            dma(out=attn_out[:, :], in_=attn_b[0:1, :])
            return

        # e vector image [128, 17]: cols 0..7 ctx, cols 8..16 emb_comb
        e_img = small.tile([128, 17], f32)
        for k in range(KH):
            prod = work.tile([128, T], f32, tag="ctx_prod")
            nc.vector.tensor_mul(prod, encT_sb[:, k * T:(k + 1) * T], attn_b)
            nc.vector.reduce_sum(out=e_img[:, k:k + 1], in_=prod, axis=AX.X)
        if stage == "A3a":
            dma(out=attn_out[:, :], in_=e_img[0:1, 0:T])
            return
        dma(out=e_img[:, 8:17], in_=ins["emb_img"][:, :])
        if stage == "A3b":
            dma(out=attn_out[:, :], in_=e_img[0:1, 8:15])
            return
        if stage == "A3":
            dma(out=z_out[:, 0:17], in_=e_img)
            return

        # ---- phase B: combine + relu -> x slice ----
        acw_sb = small.tile([128, 2176], f32)
        dma(out=acw_sb, in_=ins["ac_lhsT"][:, :])
        acb_sb = small.tile([128, 1], f32)
        dma(out=acb_sb, in_=ins["acb_col"][:, :])

        ps_x = psum.tile([128, 1], f32, tag="ps_small")
        for k in range(17):
            nc.tensor.matmul(ps_x, acw_sb[:, k * 128:(k + 1) * 128],
                             e_img[:, k:k + 1], start=(k == 0), stop=(k == 16))
        x_col = work.tile([128, 1], f32)
        nc.scalar.activation(out=x_col, in_=ps_x, func=AF.Relu, bias=acb_sb)
        if stage == "A4":
            dma(out=z_out[:, 0:1], in_=x_col)
            return

        d_xin = dma(out=cc_x_in.ap().rearrange("a p -> p a"), in_=x_col)
        cc2 = nc.gpsimd.collective_compute(
            "AllGather", ALU.bypass, replica_groups=groups,
            ins=[cc_x_in.ap()], outs=[cc_x_out.ap()])
        add_dep_helper(cc2.ins, d_xin.ins, reason="cc2 wait x dma")

        xh_sb = small.tile([128, 16], f32)
        d_xg = dma(out=xh_sb[:, 0:KH], in_=cc_x_out.ap().rearrange("k p -> p k"))
        add_dep_helper(d_xg.ins, cc2.ins, reason="x gather after cc2")
        dma(out=xh_sb[:, KH:16], in_=ins["h0_img"][:, :])

        # ---- phase C: LSTM cell ----
        if stage == "B2":
            zf = small.tile([128, VTILES], f32, tag="zf_bisect")
            nc.vector.memset(zf, 0.0)
            nc.vector.tensor_copy(zf[:, 0:16], xh_sb)
            dma(out=z_out[:, :], in_=zf)
            return

        mw_sb = small.tile([128, 16 * 512], f32)
        dma(out=mw_sb, in_=ins["m_lhsT"][:, :])
        bg_sb = small.tile([1, 512], f32)
        dma(out=bg_sb, in_=ins["bg_row"][:, :])
        c0_sb = small.tile([1, S], f32)
        dma(out=c0_sb, in_=ins["c0_row"][:, :])

        ps_g = psum.tile([1, 512], f32, tag="ps_small")
        for k in range(16):
            nc.tensor.matmul(ps_g, xh_sb[:, k:k + 1],
                             mw_sb[:, k * 512:(k + 1) * 512],
                             start=(k == 0), stop=(k == 15))
        g_sb = work.tile([1, 512], f32)
        nc.vector.tensor_add(g_sb, ps_g, bg_sb)

        i_sb = work.tile([1, S], f32, tag="gate_i")
        nc.scalar.activation(out=i_sb, in_=g_sb[:, 0:S], func=AF.Sigmoid)
        f_sb = work.tile([1, S], f32, tag="gate_f")
        nc.scalar.activation(out=f_sb, in_=g_sb[:, S:2 * S], func=AF.Sigmoid)
        gg_sb = work.tile([1, S], f32, tag="gate_g")
        nc.scalar.activation(out=gg_sb, in_=g_sb[:, 2 * S:3 * S], func=AF.Tanh)
        o_sb = work.tile([1, S], f32, tag="gate_o")
        nc.scalar.activation(out=o_sb, in_=g_sb[:, 3 * S:4 * S], func=AF.Sigmoid)

        t1 = work.tile([1, S], f32, tag="lstm_t1")
        nc.vector.tensor_mul(t1, f_sb, c0_sb)
        t2 = work.tile([1, S], f32, tag="lstm_t2")
        nc.vector.tensor_mul(t2, i_sb, gg_sb)
        c1_sb = work.tile([1, S], f32, tag="lstm_c1")
        nc.vector.tensor_add(c1_sb, t1, t2)
        tc1 = work.tile([1, S], f32, tag="lstm_tc1")
        nc.scalar.activation(out=tc1, in_=c1_sb, func=AF.Tanh)
        h1_row = work.tile([1, S], f32, tag="lstm_h1")
        nc.vector.tensor_mul(h1_row, o_sb, tc1)

        dma(out=c1_out[:, :], in_=c1_sb)
        dma(out=h1_out[:, :], in_=h1_row)
        if stage == "C1":
            return

        d_hin = dma(out=cc_h_in[:, :], in_=h1_row)
        cc3 = nc.gpsimd.collective_compute(
            "AllGather", ALU.bypass, replica_groups=groups,
            ins=[cc_h_in.ap()], outs=[cc_h_out.ap()])
        add_dep_helper(cc3.ins, d_hin.ins, reason="cc3 wait h1 dma")
        h1_img = small.tile([128, KH], f32)
        d_hg = dma(out=h1_img, in_=cc_h_out.ap().rearrange("k p -> p k"))
        add_dep_helper(d_hg.ins, cc3.ins, reason="h1 gather after cc3")

        if stage == "C":
            return

        # ---- phase D: vocab shard matvec + log_softmax ----
        outb_sb = small.tile([128, VTILES], f32)
        dma(out=outb_sb, in_=ins["outb_img"][:, :])
        z_sb = small.tile([128, VTILES], f32)

        for c in range(D_NCHUNKS):
            w_sb = wpool.tile([128, D_CHUNK_COLS], f32, tag="wchunk")
            dma(out=w_sb, in_=ins["w_img"][:, c * D_CHUNK_COLS:(c + 1) * D_CHUNK_COLS])
            ps_z = psz_pool.tile([128, D_CHUNK_VT], f32, tag="ps_z")
            for vt in range(D_CHUNK_VT):
                for k in range(KH):
                    col = (vt * KH + k) * 128
                    nc.tensor.matmul(ps_z[:, vt:vt + 1],
                                     w_sb[:, col:col + 128],
                                     h1_img[:, k:k + 1],
                                     start=(k == 0), stop=(k == KH - 1))
            nc.vector.tensor_add(z_sb[:, c * D_CHUNK_VT:(c + 1) * D_CHUNK_VT],
                                 ps_z, outb_sb[:, c * D_CHUNK_VT:(c + 1) * D_CHUNK_VT])

        # sum(exp(z)) over this core's shard (z is small, no max shift needed)
        ze_sb = small.tile([128, VTILES], f32)
        sexp = work.tile([128, 1], f32)
        nc.scalar.activation(out=ze_sb, in_=z_sb, func=AF.Exp, accum_out=sexp)
        ones_col = small.tile([128, 1], f32)
        nc.vector.memset(ones_col, 1.0)
        ps_S = psum.tile([1, 1], f32, tag="ps_small")
        nc.tensor.matmul(ps_S, sexp, ones_col, start=True, stop=True)

        S_row = work.tile([1, 8], f32, tag="srow")
        nc.vector.memset(S_row, 0.0)
        nc.vector.tensor_copy(S_row[:, 0:1], ps_S)
        d_zin = dma(out=cc_z_in[:, :], in_=S_row)
        cc4 = nc.gpsimd.collective_compute(
            "AllReduce", ALU.add, replica_groups=groups,
            ins=[cc_z_in.ap()], outs=[cc_z_out.ap()])
        add_dep_helper(cc4.ins, d_zin.ins, reason="cc4 wait sumexp dma")
        Sg_row = work.tile([1, 8], f32, tag="sgrow")
        d_zg = dma(out=Sg_row, in_=cc_z_out[:, :])
        add_dep_helper(d_zg.ins, cc4.ins, reason="sumexp gather after cc4")

        logZ = work.tile([1, 1], f32, tag="logz")
        nc.scalar.activation(out=logZ, in_=Sg_row[:, 0:1], func=AF.Ln)
        neglogZ = work.tile([1, 1], f32, tag="neglogz")
        nc.vector.tensor_scalar_mul(neglogZ, logZ, -1.0)
        ps_nl = psum.tile([128, 1], f32, tag="ps_small")
        nc.tensor.matmul(ps_nl, ones_row, neglogZ, start=True, stop=True)
        nl_col = work.tile([128, 1], f32, tag="nlcol")
        nc.vector.tensor_copy(nl_col, ps_nl)

        zf_sb = small.tile([128, VTILES], f32)
        nc.vector.tensor_scalar_add(zf_sb, z_sb, nl_col)
        dma(out=z_out[:, :], in_=zf_sb)


def get_module():
    global _module_cache
    if _module_cache is None:
        _module_cache = build_module()
    return _module_cache


def assemble(results: list[dict]) -> tuple:
    out = np.empty((1, V), dtype=np.float32)
    h1 = np.empty(H, dtype=np.float32)
    c1 = np.empty(H, dtype=np.float32)
    for j in range(NCORES):
        z2d = results[j]["z_out"]                     # [128, 49]
        out[0, j * VS:(j + 1) * VS] = z2d.T.reshape(-1)[:VS]
        h1[j * S:(j + 1) * S] = results[j]["h1_out"].reshape(-1)
        c1[j * S:(j + 1) * S] = results[j]["c1_out"].reshape(-1)
    attn = results[0]["attn_out"].reshape(1, T).astype(np.float32)
    return out, h1.reshape(1, 1, H), c1.reshape(1, 1, H), attn


def run(inputs: dict, trace: bool = False):
    from concourse import bass_utils
    nc = get_module()
    in_maps = prep_in_maps(inputs)
    res = bass_utils.run_bass_kernel_spmd(
        nc, in_maps, core_ids=list(range(NCORES)), trace=trace)
    return assemble(res.results), res


def kernel(**inputs):
    (out, h1, c1, attn), _ = run(inputs)
    return out, h1, c1, attn
